# revision 6
# baseline (speedup 1.0000x reference)
import sys

sys.path.insert(0, "/opt/trn_rl_repo")

import numpy as np

import concourse.bass as bass  # noqa: F401
import concourse.tile as tile
from concourse import bacc, bass_utils, mybir

F32 = mybir.dt.float32
AF = mybir.ActivationFunctionType
ALU = mybir.AluOpType

E, H, HD, FFD, G, A, BFULL = 128, 8, 16, 512, 1000, 100000, 65536
NCORES = 8
NPC = BFULL // NCORES  # 8192 rows per core
BT = 512               # batch tile (samples per inner iteration)
NT = NPC // BT         # 16
NL = 6
EPS = 1e-5

_CACHE = {}


def _declare_inputs(nc):
    """Declare all dram tensors; returns dict name -> AP."""
    d = {}

    def din(name, shape):
        d[name] = nc.dram_tensor(name, list(shape), F32, kind="ExternalInput").ap()

    # per-core data (transposed: features on partitions)
    din("sgT", (E, NPC))
    din("saT", (E, NPC))
    din("ugT", (E, NPC))
    din("uaT", (E, NPC))
    din("numT", (3, NPC))
    # numeric MLP
    din("np_w1", (3, 64)); din("np_b1", (64, 1)); din("np_g1", (64, 1)); din("np_t1", (64, 1))
    din("np_w2", (64, E)); din("np_b2", (E, 1)); din("np_g2", (E, 1)); din("np_t2", (E, 1))
    # fusion
    din("sf_w", (3 * E, E)); din("sf_b", (E, 1)); din("sf_g", (E, 1)); din("sf_t", (E, 1))
    din("uf_w", (2 * E, E)); din("uf_b", (E, 1)); din("uf_g", (E, 1)); din("uf_t", (E, 1))
    # transformer layers
    for i in range(NL):
        p = f"l{i}_"
        din(p + "qkv_w", (E, 3 * E)); din(p + "qkv_b", (3 * E, 1))
        din(p + "proj_w", (E, E)); din(p + "proj_b", (E, 1))
        din(p + "g1", (E, 1)); din(p + "t1", (E, 1))
        din(p + "ff1", (E, FFD)); din(p + "fb1", (FFD, 1))
        din(p + "ff2", (FFD, E)); din(p + "fb2", (E, 1))
        din(p + "g2", (E, 1)); din(p + "t2", (E, 1))
    # head
    din("sp_w", (E, E)); din("sp_b", (E, 1)); din("sp_g", (E, 1)); din("sp_t", (E, 1))
    din("up_w", (E, E)); din("up_b", (E, 1)); din("up_g", (E, 1)); din("up_t", (E, 1))
    din("cav_w", (E, E)); din("cav_b", (E, 1))
    din("cao_w", (E, E)); din("cao_b", (E, 1))
    din("cn_g", (E, 1)); din("cn_t", (E, 1))
    din("pr_w1", (2 * E, 2 * E)); din("pr_b1", (2 * E, 1)); din("pr_g1", (2 * E, 1)); din("pr_t1", (2 * E, 1))
    din("pr_w2", (2 * E, E)); din("pr_b2", (E, 1)); din("pr_g2", (E, 1)); din("pr_t2", (E, 1))
    din("pr_w3", (E, 64)); din("pr_b3", (64, 1)); din("pr_g3", (64, 1)); din("pr_t3", (64, 1))
    din("pr_w4", (64, 1)); din("pr_b4", (1, 1))
    din("headmaskT", (H, E)); din("hm", (E, H))
    d["out"] = nc.dram_tensor("out", [1, NPC], F32, kind="ExternalOutput").ap()
    return d


def _build_program():
    nc = bacc.Bacc("TRN2", target_bir_lowering=False, debug=False, num_devices=NCORES)
    for val in (EPS, 0.25, -1.0, -1.0 / 64, -1.0 / 128, -1.0 / 256,
                1.0 / 64, 1.0 / 128, 1.0 / 256):
        t = nc.alloc_sbuf_tensor(f"constx_{len(nc.const_aps.aps)}", [128, 1], F32)
        nc.gpsimd.memset(t.ap(), val)
        nc.const_aps.aps[(F32, val)] = t.ap()
    nc.all_engine_barrier()
    d = _declare_inputs(nc)

    from contextlib import ExitStack
    with tile.TileContext(nc) as tc, ExitStack() as es:
        wpool = es.enter_context(tc.tile_pool(name="weights", bufs=1))
        apool = es.enter_context(tc.tile_pool(name="act", bufs=1))
        dpool = es.enter_context(tc.tile_pool(name="data", bufs=2))
        xpool = es.enter_context(tc.tile_pool(name="xp", bufs=2))
        psum = es.enter_context(tc.tile_pool(name="ps", bufs=1, space="PSUM"))

        W = {}

        def wload(name, src_ap, shape):
            t = wpool.tile(list(shape), F32, tag=name)
            nc.sync.dma_start(t[:], src_ap)
            W[name] = t

        # simple whole-tensor weights
        for name, ap in d.items():
            if name in ("out", "sgT", "saT", "ugT", "uaT", "numT"):
                continue
            sh = ap.shape
            if sh[0] <= 128:
                wload(name, ap[:], sh)
            else:
                # split rows into 128-chunks
                nch = sh[0] // 128
                for k in range(nch):
                    wload(f"{name}_k{k}", ap[k * 128:(k + 1) * 128, :], (128, sh[1]))

        # constants
        ones_col = wpool.tile([128, 1], F32, tag="ones_col")
        nc.vector.memset(ones_col[:], 1.0)
        ones_row = wpool.tile([1, 128], F32, tag="ones_row")
        nc.vector.memset(ones_row[:], 1.0)

        def MM(psname, lhsT, rhs, m):
            pstile = psum.tile([E, BT], F32, tag=psname)
            nc.tensor.matmul(pstile[0:m, :], lhsT, rhs, start=True, stop=True)
            return pstile

        def ln_apply(xs_feats, gs, ts, outs, gelu, tag):
            """xs_feats: list of (ap [f,BT], f). LN across total features, then
            per-chunk affine (+ optional gelu) into outs."""
            F = sum(f for _, f in xs_feats)
            s_ps = psum.tile([1, BT], F32, tag="lns")
            q_ps = psum.tile([1, BT], F32, tag="lnq")
            n = len(xs_feats)
            for j, (x, f) in enumerate(xs_feats):
                st, sp = (j == 0), (j == n - 1)
                nc.tensor.matmul(s_ps[0:1, :], ones_col[0:f, :], x, start=st, stop=sp)
                sq = apool.tile([128, BT], F32, tag="ln_sq")
                nc.scalar.activation(sq[0:f, :], x, AF.Square)
                nc.tensor.matmul(q_ps[0:1, :], ones_col[0:f, :], sq[0:f, :], start=st, stop=sp)
            nm = apool.tile([1, BT], F32, tag="ln_nm")
            nc.scalar.mul(nm[:], s_ps[0:1, :], -1.0 / F)
            msq = apool.tile([1, BT], F32, tag="ln_msq")
            nc.scalar.activation(msq[:], nm[:], AF.Square)
            var = apool.tile([1, BT], F32, tag="ln_var")
            nc.vector.scalar_tensor_tensor(var[:], q_ps[0:1, :], 1.0 / F, msq[:], ALU.mult, ALU.subtract)
            std = apool.tile([1, BT], F32, tag="ln_std")
            nc.scalar.activation(std[:], var[:], AF.Sqrt, bias=EPS)
            rstd = apool.tile([1, BT], F32, tag="ln_rstd")
            nc.vector.reciprocal(rstd[:], std[:])
            cc = apool.tile([1, BT], F32, tag="ln_cc")
            nc.vector.tensor_mul(cc[:], nm[:], rstd[:])
            fmax = max(f for _, f in xs_feats)
            A_ps = psum.tile([E, BT], F32, tag="lnA")
            nc.tensor.matmul(A_ps[0:fmax, :], ones_row[:, 0:fmax], rstd[:])
            C_ps = psum.tile([E, BT], F32, tag="lnC")
            nc.tensor.matmul(C_ps[0:fmax, :], ones_row[:, 0:fmax], cc[:])
            for (x, f), g, t, out in zip(xs_feats, gs, ts, outs):
                t1 = apool.tile([128, BT], F32, tag="ln_t1")
                nc.vector.tensor_mul(t1[0:f, :], x, A_ps[0:f, :])
                t2 = apool.tile([128, BT], F32, tag="ln_t2")
                nc.vector.tensor_add(t2[0:f, :], t1[0:f, :], C_ps[0:f, :])
                nc.scalar.activation(out, t2[0:f, :], AF.Gelu if gelu else AF.Identity,
                                     bias=t, scale=g)

        for it in range(NT):
            cs = slice(it * BT, (it + 1) * BT)
            sg = dpool.tile([E, BT], F32, tag="sg"); nc.sync.dma_start(sg[:], d["sgT"][:, cs])
            sa = dpool.tile([E, BT], F32, tag="sa"); nc.sync.dma_start(sa[:], d["saT"][:, cs])
            ug = dpool.tile([E, BT], F32, tag="ug"); nc.sync.dma_start(ug[:], d["ugT"][:, cs])
            ua = dpool.tile([E, BT], F32, tag="ua"); nc.sync.dma_start(ua[:], d["uaT"][:, cs])
            nm_in = dpool.tile([3, BT], F32, tag="nm"); nc.sync.dma_start(nm_in[:], d["numT"][:, cs])

            # ---- numeric MLP ----
            ps = MM("mm", W["np_w1"][:], nm_in[:], 64)
            n1 = apool.tile([64, BT], F32, tag="n1")
            nc.scalar.activation(n1[:], ps[0:64, :], AF.Identity, bias=W["np_b1"][:])
            n1g = apool.tile([64, BT], F32, tag="n1g")
            ln_apply([(n1[:], 64)], [W["np_g1"][:]], [W["np_t1"][:]], [n1g[:]], True, "ln_np1")
            ps = MM("mm", W["np_w2"][:], n1g[:], E)
            n2 = apool.tile([E, BT], F32, tag="n2")
            nc.scalar.activation(n2[:], ps[0:E, :], AF.Identity, bias=W["np_b2"][:])
            numv = apool.tile([E, BT], F32, tag="numv")
            ln_apply([(n2[:], E)], [W["np_g2"][:]], [W["np_t2"][:]], [numv[:]], True, "ln_np2")

            # ---- fusion -> x [E, 2*BT] (token0 | token1) ----
            x = xpool.tile([E, 2 * BT], F32, tag="x")
            ps = psum.tile([E, BT], F32, tag="mm")
            nc.tensor.matmul(ps[:], W["sf_w_k0"][:], sg[:], start=True, stop=False)
            nc.tensor.matmul(ps[:], W["sf_w_k1"][:], sa[:], start=False, stop=False)
            nc.tensor.matmul(ps[:], W["sf_w_k2"][:], numv[:], start=False, stop=True)
            scat = apool.tile([E, BT], F32, tag="scat")
            nc.scalar.activation(scat[:], ps[:], AF.Identity, bias=W["sf_b"][:])
            ln_apply([(scat[:], E)], [W["sf_g"][:]], [W["sf_t"][:]], [x[:, 0:BT]], True, "ln_sf")
            ps = psum.tile([E, BT], F32, tag="mm")
            nc.tensor.matmul(ps[:], W["uf_w_k0"][:], ug[:], start=True, stop=False)
            nc.tensor.matmul(ps[:], W["uf_w_k1"][:], ua[:], start=False, stop=True)
            ucat = apool.tile([E, BT], F32, tag="ucat")
            nc.scalar.activation(ucat[:], ps[:], AF.Identity, bias=W["uf_b"][:])
            ln_apply([(ucat[:], E)], [W["uf_g"][:]], [W["uf_t"][:]], [x[:, BT:2 * BT]], True, "ln_uf")

            # ---- transformer layers ----
            for li in range(NL):
                p = f"l{li}_"
                qkvw = W[p + "qkv_w"]
                qkvb = W[p + "qkv_b_k0"], W[p + "qkv_b_k1"], W[p + "qkv_b_k2"]
                q = apool.tile([E, 2 * BT], F32, tag="q")
                k = apool.tile([E, 2 * BT], F32, tag="k")
                v = apool.tile([E, 2 * BT], F32, tag="v")
                for tok in range(2):
                    ts_ = slice(tok * BT, (tok + 1) * BT)
                    for j, dst in enumerate((q, k, v)):
                        ps = MM("mm", qkvw[:, j * E:(j + 1) * E], x[:, ts_], E)
                        nc.scalar.activation(dst[:, ts_], ps[0:E, :], AF.Identity, bias=qkvb[j][:])

                # scores: [s00|s01|s10|s11]
                prod = apool.tile([E, 4 * BT], F32, tag="big4")
                nc.vector.tensor_mul(prod[:, 0:BT], q[:, 0:BT], k[:, 0:BT])
                nc.vector.tensor_mul(prod[:, BT:2 * BT], q[:, 0:BT], k[:, BT:2 * BT])
                nc.vector.tensor_mul(prod[:, 2 * BT:3 * BT], q[:, BT:2 * BT], k[:, 0:BT])
                nc.vector.tensor_mul(prod[:, 3 * BT:4 * BT], q[:, BT:2 * BT], k[:, BT:2 * BT])
                sc = apool.tile([H, 4 * BT], F32, tag="sc")
                for j in range(4):
                    ps = psum.tile([H, BT], F32, tag="sc")
                    nc.tensor.matmul(ps[0:H, :], W["hm"][:], prod[:, j * BT:(j + 1) * BT], start=True, stop=True)
                    nc.vector.tensor_copy(sc[:, j * BT:(j + 1) * BT], ps[0:H, :])
                dsc = apool.tile([H, 2 * BT], F32, tag="dsc")
                nc.vector.tensor_sub(dsc[:, 0:BT], sc[:, 0:BT], sc[:, BT:2 * BT])
                nc.vector.tensor_sub(dsc[:, BT:2 * BT], sc[:, 2 * BT:3 * BT], sc[:, 3 * BT:4 * BT])
                pp = apool.tile([H, 4 * BT], F32, tag="pp")
                nc.scalar.activation(pp[:, 0:BT], dsc[:, 0:BT], AF.Sigmoid, scale=float(HD ** -0.5))
                nc.scalar.activation(pp[:, 2 * BT:3 * BT], dsc[:, BT:2 * BT], AF.Sigmoid, scale=float(HD ** -0.5))
                nc.vector.tensor_scalar(pp[:, BT:2 * BT], pp[:, 0:BT], -1.0, 1.0, ALU.mult, ALU.add)
                nc.vector.tensor_scalar(pp[:, 3 * BT:4 * BT], pp[:, 2 * BT:3 * BT], -1.0, 1.0, ALU.mult, ALU.add)

                ao = apool.tile([E, 2 * BT], F32, tag="ao")
                for tok in range(2):
                    t_a = apool.tile([E, BT], F32, tag="avA")
                    t_b = apool.tile([E, BT], F32, tag="avB")
                    ps = psum.tile([E, BT], F32, tag="bc")
                    nc.tensor.matmul(ps[:], W["headmaskT"][:], pp[:, 2 * tok * BT:(2 * tok + 1) * BT], start=True, stop=True)
                    nc.vector.tensor_mul(t_a[:], ps[:], v[:, 0:BT])
                    ps2 = psum.tile([E, BT], F32, tag="bc2")
                    nc.tensor.matmul(ps2[:], W["headmaskT"][:], pp[:, (2 * tok + 1) * BT:(2 * tok + 2) * BT], start=True, stop=True)
                    nc.vector.tensor_mul(t_b[:], ps2[:], v[:, BT:2 * BT])
                    nc.vector.tensor_add(ao[:, tok * BT:(tok + 1) * BT], t_a[:], t_b[:])

                # proj + residual + LN1
                xr = xpool.tile([E, 2 * BT], F32, tag="xr")
                for tok in range(2):
                    ts_ = slice(tok * BT, (tok + 1) * BT)
                    ps = MM("mm", W[p + "proj_w"][:], ao[:, ts_], E)
                    ttmp = apool.tile([E, BT], F32, tag="ptmp")
                    nc.scalar.activation(ttmp[:], ps[0:E, :], AF.Identity, bias=W[p + "proj_b"][:])
                    rr = apool.tile([E, BT], F32, tag="rr")
                    nc.vector.tensor_add(rr[:], ttmp[:], x[:, ts_])
                    ln_apply([(rr[:], E)], [W[p + "g1"][:]], [W[p + "t1"][:]], [xr[:, ts_]], False, "ln_a")

                # FF + residual + LN2
                x2 = xpool.tile([E, 2 * BT], F32, tag="x")
                for tok in range(2):
                    ts_ = slice(tok * BT, (tok + 1) * BT)
                    hh = apool.tile([E, 4 * BT], F32, tag="big4")
                    for j in range(4):
                        ps = MM("mm", W[p + "ff1"][:, j * E:(j + 1) * E], xr[:, ts_], E)
                        nc.scalar.activation(hh[:, j * BT:(j + 1) * BT], ps[0:E, :], AF.Gelu,
                                             bias=W[p + f"fb1_k{j}"][:])
                    ps2 = psum.tile([E, BT], F32, tag="mm")
                    for j in range(4):
                        nc.tensor.matmul(ps2[:], W[p + f"ff2_k{j}"][:], hh[:, j * BT:(j + 1) * BT],
                                         start=(j == 0), stop=(j == 3))
                    ftmp = apool.tile([E, BT], F32, tag="ftmp")
                    nc.scalar.activation(ftmp[:], ps2[:], AF.Identity, bias=W[p + "fb2"][:])
                    rr2 = apool.tile([E, BT], F32, tag="rr2")
                    nc.vector.tensor_add(rr2[:], ftmp[:], xr[:, ts_])
                    ln_apply([(rr2[:], E)], [W[p + "g2"][:]], [W[p + "t2"][:]], [x2[:, ts_]], False, "ln_b")
                x = x2

            # ---- head ----
            ps = MM("mm", W["sp_w"][:], x[:, 0:BT], E)
            spt_ = apool.tile([E, BT], F32, tag="spt_")
            nc.scalar.activation(spt_[:], ps[0:E, :], AF.Identity, bias=W["sp_b"][:])
            spt = apool.tile([E, BT], F32, tag="spt")
            ln_apply([(spt_[:], E)], [W["sp_g"][:]], [W["sp_t"][:]], [spt[:]], True, "ln_sp")
            ps = MM("mm", W["up_w"][:], x[:, BT:2 * BT], E)
            upt_ = apool.tile([E, BT], F32, tag="upt_")
            nc.scalar.activation(upt_[:], ps[0:E, :], AF.Identity, bias=W["up_b"][:])
            upt = apool.tile([E, BT], F32, tag="upt")
            ln_apply([(upt_[:], E)], [W["up_g"][:]], [W["up_t"][:]], [upt[:]], True, "ln_up")

            ps = MM("mm", W["cav_w"][:], spt[:], E)
            vp = apool.tile([E, BT], F32, tag="vp")
            nc.scalar.activation(vp[:], ps[0:E, :], AF.Identity, bias=W["cav_b"][:])
            ps = MM("mm", W["cao_w"][:], vp[:], E)
            cot = apool.tile([E, BT], F32, tag="cot")
            nc.scalar.activation(cot[:], ps[0:E, :], AF.Identity, bias=W["cao_b"][:])
            co2 = apool.tile([E, BT], F32, tag="co2")
            nc.vector.tensor_add(co2[:], cot[:], upt[:])
            con = apool.tile([E, BT], F32, tag="con")
            ln_apply([(co2[:], E)], [W["cn_g"][:]], [W["cn_t"][:]], [con[:]], False, "ln_cn")

            # c = [spt; con] (256 features)
            h1 = apool.tile([E, 2 * BT], F32, tag="h1")
            for m in range(2):
                ps = psum.tile([E, BT], F32, tag="mm")
                nc.tensor.matmul(ps[:], W["pr_w1_k0"][:, m * E:(m + 1) * E], spt[:], start=True, stop=False)
                nc.tensor.matmul(ps[:], W["pr_w1_k1"][:, m * E:(m + 1) * E], con[:], start=False, stop=True)
                nc.scalar.activation(h1[:, m * BT:(m + 1) * BT], ps[:], AF.Identity, bias=W[f"pr_b1_k{m}"][:])
            h1g = apool.tile([E, 2 * BT], F32, tag="h1g")
            ln_apply([(h1[:, 0:BT], E), (h1[:, BT:2 * BT], E)],
                     [W["pr_g1_k0"][:], W["pr_g1_k1"][:]],
                     [W["pr_t1_k0"][:], W["pr_t1_k1"][:]],
                     [h1g[:, 0:BT], h1g[:, BT:2 * BT]], True, "ln_p1")
            ps = psum.tile([E, BT], F32, tag="mm")
            nc.tensor.matmul(ps[:], W["pr_w2_k0"][:], h1g[:, 0:BT], start=True, stop=False)
            nc.tensor.matmul(ps[:], W["pr_w2_k1"][:], h1g[:, BT:2 * BT], start=False, stop=True)
            h2 = apool.tile([E, BT], F32, tag="h2")
            nc.scalar.activation(h2[:], ps[:], AF.Identity, bias=W["pr_b2"][:])
            h2g = apool.tile([E, BT], F32, tag="h2g")
            ln_apply([(h2[:], E)], [W["pr_g2"][:]], [W["pr_t2"][:]], [h2g[:]], True, "ln_p2")
            ps = MM("mm", W["pr_w3"][:], h2g[:], 64)
            h3 = apool.tile([64, BT], F32, tag="h3")
            nc.scalar.activation(h3[:], ps[0:64, :], AF.Identity, bias=W["pr_b3"][:])
            h3g = apool.tile([64, BT], F32, tag="h3g")
            ln_apply([(h3[:], 64)], [W["pr_g3"][:]], [W["pr_t3"][:]], [h3g[:]], True, "ln_p3")
            ps = MM("mm", W["pr_w4"][:], h3g[:], 1)
            ot = apool.tile([1, BT], F32, tag="ot")
            nc.scalar.activation(ot[:], ps[0:1, :], AF.Sigmoid, bias=W["pr_b4"][:])
            nc.sync.dma_start(d["out"][0:1, cs], ot[:])

    nc.compile()
    return nc


def _prep_weight_maps(params):
    P = {k: np.asarray(v, dtype=np.float32) for k, v in params.items() if k != "layers"}
    L = [{k: np.asarray(v, dtype=np.float32) for k, v in lp.items()} for lp in params["layers"]]
    w = {}

    def col(x):
        return np.ascontiguousarray(np.asarray(x, np.float32).reshape(-1, 1))

    w["np_w1"] = P["np_w1"]; w["np_b1"] = col(P["np_b1"]); w["np_g1"] = col(P["np_ln1g"]); w["np_t1"] = col(P["np_ln1b"])
    w["np_w2"] = P["np_w2"]; w["np_b2"] = col(P["np_b2"]); w["np_g2"] = col(P["np_ln2g"]); w["np_t2"] = col(P["np_ln2b"])
    w["sf_w"] = P["sf_w"]; w["sf_b"] = col(P["sf_b"]); w["sf_g"] = col(P["sf_lng"]); w["sf_t"] = col(P["sf_lnb"])
    w["uf_w"] = P["uf_w"]; w["uf_b"] = col(P["uf_b"]); w["uf_g"] = col(P["uf_lng"]); w["uf_t"] = col(P["uf_lnb"])
    for i, lp in enumerate(L):
        p = f"l{i}_"
        w[p + "qkv_w"] = lp["qkv_w"]; w[p + "qkv_b"] = col(lp["qkv_b"])
        w[p + "proj_w"] = lp["proj_w"]; w[p + "proj_b"] = col(lp["proj_b"])
        w[p + "g1"] = col(lp["ln1_g"]); w[p + "t1"] = col(lp["ln1_b"])
        w[p + "ff1"] = lp["ff_w1"]; w[p + "fb1"] = col(lp["ff_b1"])
        w[p + "ff2"] = lp["ff_w2"]; w[p + "fb2"] = col(lp["ff_b2"])
        w[p + "g2"] = col(lp["ln2_g"]); w[p + "t2"] = col(lp["ln2_b"])
    w["sp_w"] = P["sp_w"]; w["sp_b"] = col(P["sp_b"]); w["sp_g"] = col(P["sp_lng"]); w["sp_t"] = col(P["sp_lnb"])
    w["up_w"] = P["up_w"]; w["up_b"] = col(P["up_b"]); w["up_g"] = col(P["up_lng"]); w["up_t"] = col(P["up_lnb"])
    w["cav_w"] = np.ascontiguousarray(P["ca_in_w"][:, 2 * E:]); w["cav_b"] = col(P["ca_in_b"][2 * E:])
    w["cao_w"] = P["ca_out_w"]; w["cao_b"] = col(P["ca_out_b"])
    w["cn_g"] = col(P["cn_g"]); w["cn_t"] = col(P["cn_b"])
    w["pr_w1"] = P["pr_w1"]; w["pr_b1"] = col(P["pr_b1"]); w["pr_g1"] = col(P["pr_ln1g"]); w["pr_t1"] = col(P["pr_ln1b"])
    w["pr_w2"] = P["pr_w2"]; w["pr_b2"] = col(P["pr_b2"]); w["pr_g2"] = col(P["pr_ln2g"]); w["pr_t2"] = col(P["pr_ln2b"])
    w["pr_w3"] = P["pr_w3"]; w["pr_b3"] = col(P["pr_b3"]); w["pr_g3"] = col(P["pr_ln3g"]); w["pr_t3"] = col(P["pr_ln3b"])
    w["pr_w4"] = P["pr_w4"]; w["pr_b4"] = col(P["pr_b4"])
    hm = np.zeros((E, H), np.float32)
    for dd in range(E):
        hm[dd, dd // HD] = 1.0
    w["hm"] = hm
    w["headmaskT"] = np.ascontiguousarray(hm.T)
    return w


def kernel(song_genre, song_artist, user_genre, user_artist, song_wpm,
           song_duration, song_tokens, params):
    if "nc" not in _CACHE:
        _CACHE["nc"] = _build_program()
    nc = _CACHE["nc"]

    w = _prep_weight_maps(params)
    ge = np.asarray(params["genre_emb"], np.float32)
    ae = np.asarray(params["artist_emb"], np.float32)
    sgi = np.asarray(song_genre).astype(np.int64)
    sai = np.asarray(song_artist).astype(np.int64)
    ugi = np.asarray(user_genre).astype(np.int64)
    uai = np.asarray(user_artist).astype(np.int64)
    sg = ge[sgi]; sa = ae[sai]; ug = ge[ugi]; ua = ae[uai]
    num = np.stack([np.asarray(song_wpm, np.float32),
                    np.asarray(song_duration, np.float32),
                    np.asarray(song_tokens, np.float32)], axis=0)  # [3, B]

    in_maps = []
    for c in range(NCORES):
        rs = slice(c * NPC, (c + 1) * NPC)
        m = dict(w)
        m["sgT"] = np.ascontiguousarray(sg[rs].T)
        m["saT"] = np.ascontiguousarray(sa[rs].T)
        m["ugT"] = np.ascontiguousarray(ug[rs].T)
        m["uaT"] = np.ascontiguousarray(ua[rs].T)
        m["numT"] = np.ascontiguousarray(num[:, rs])
        in_maps.append(m)

    res = bass_utils.run_bass_kernel_spmd(nc, in_maps, core_ids=list(range(NCORES)))
    out = np.concatenate([np.asarray(res.results[c]["out"]).reshape(-1) for c in range(NCORES)])
    return out.astype(np.float32)
